# revision 2
# baseline (speedup 1.0000x reference)
"""CrossAndCompress Trainium2 kernel (fp16-wire, PE-dot version).

Reference computation (per row r of the batch):
    a_r = enc_item[r] . theta_vv        b_r = enc_user[r] . theta_ev
    c_r = enc_item[r] . theta_ve        d_r = enc_user[r] . theta_ee
    v_out[r] = enc_user[r] * a_r + enc_item[r] * b_r + beta_v
    e_out[r] = enc_user[r] * c_r + enc_item[r] * d_r + beta_e

Sharding: pure data parallel — batch dim (16384) split across 8 NeuronCores
(2048 rows each); theta/beta replicated.

The correctness gate is rel_err < 2e-2, which admits a 16-bit wire: the host
casts inputs to fp16 and the device writes fp16 outputs (upcast on the host).
That halves HBM traffic to ~17MB/core (~46us at the ~360GB/s per-core DMA
roofline) vs ~36MB (~97us) for fp32 — the fp32 baseline could never beat 97us.

Per-core pipeline over 16 tiles of [128 rows x 2048] (u|it packed on the free
axis so each tile is ONE dma in / ONE dma out / ONE xbar transpose — HWDGE
triggers serialize ~0.6us each through a shared resource, so trigger count
is a first-order cost):
  - DMA in packed tile xt = [u | it] (fp16)
  - dma_start_transpose xt -> xp [128, 16, 128] on the DMA xbar
    (xp[p,s,j] = xt[j, s*128+p]; no compute-engine time)
  - TensorE: 16 small matmuls xp-chunk @ theta-chunk accumulating the four
    dots (b,d | a,c) into a [128,4] PSUM tile — dots leave the Vector engine
    entirely (the fp32 baseline spent 4 of its 6 DVE passes on them; DVE has
    no fp16 fast mode for mul-reduce ops, so 16-bit alone wouldn't help)
  - DVE: tiny [128,4] PSUM->SBUF dots copy (ACT scale APs must live in SBUF)
  - ScalarE: p2 = it*b, p4 = it*d (activation Copy with per-partition scale)
  - DVE: v = u*a + p2, e = u*c + p4 (scalar_tensor_tensor), fp16 out
  - DMA out packed [v | e]
Engine budgets/core: DMA ~46us (bound), DVE ~46us, ACT ~40us, PE ~14us.
"""

import numpy as np

B, D = 16384, 1024
N_CORES = 8
ROWS_PER_CORE = B // N_CORES  # 2048
TILE_P = 128
N_TILES = ROWS_PER_CORE // TILE_P  # 16
N_CHUNKS = D // TILE_P  # 8

_PROGRAM_CACHE: dict = {}


def _build_program(with_beta: bool):
    import concourse.mybir as mybir
    import concourse.tile as tile
    from concourse import bacc

    f16 = mybir.dt.float16
    f32 = mybir.dt.float32
    AF = mybir.ActivationFunctionType
    OP = mybir.AluOpType

    nc = bacc.Bacc(
        "TRN2",
        target_bir_lowering=False,
        debug=False,
        enable_asserts=False,
        num_devices=N_CORES,
    )

    # xin[r, 0:D] = enc_user row, xin[r, D:2D] = enc_item row (fp16)
    xin_h = nc.dram_tensor(
        "xin", [ROWS_PER_CORE, 2 * D], f16, kind="ExternalInput"
    ).ap()
    # th_pe[p, s, :]: s<8 -> (t_ev, t_ee) chunk s; s>=8 -> (t_vv, t_ve) chunk s-8
    th_h = nc.dram_tensor("th_pe", [TILE_P, 2 * N_CHUNKS, 2], f16,
                          kind="ExternalInput").ap()
    if with_beta:
        be_h = nc.dram_tensor("betas", [TILE_P, 2, D], f16,
                              kind="ExternalInput").ap()
    # xout[r, 0, :] = v_out row, xout[r, 1, :] = e_out row (fp16)
    xout_h = nc.dram_tensor(
        "xout", [ROWS_PER_CORE, 2, D], f16, kind="ExternalOutput"
    ).ap()

    with tile.TileContext(nc) as tc:
        with (
            tc.tile_pool(name="const", bufs=1) as cpool,
            tc.tile_pool(name="io", bufs=3) as io,
            tc.tile_pool(name="xpose", bufs=2) as xpose,
            tc.tile_pool(name="out", bufs=2) as outp,
            tc.tile_pool(name="work", bufs=2) as work,
            tc.tile_pool(name="dots", bufs=2, space="PSUM") as dpool,
        ):
            th = cpool.tile([TILE_P, 2 * N_CHUNKS, 2], f16, tag="th")
            nc.sync.dma_start(th[:], th_h[:, :, :])
            if with_beta:
                betas = cpool.tile([TILE_P, 2, D], f16, tag="betas")
                nc.sync.dma_start(betas[:], be_h[:, :, :])

            for i in range(N_TILES):
                rows = slice(i * TILE_P, (i + 1) * TILE_P)
                xt = io.tile([TILE_P, 2 * D], f16, tag="xt")
                nc.sync.dma_start(xt[:], xin_h[rows, :])
                u = xt[:, 0:D]
                it = xt[:, D : 2 * D]

                # xbar transpose: xp[p, s, j] = xt[j, s*128+p]
                xp = xpose.tile([TILE_P, 2 * N_CHUNKS, TILE_P], f16, tag="xp")
                nc.sync.dma_start_transpose(xp[:], xt[:])

                # dots[:, 0]=b=u.t_ev  [:,1]=d=u.t_ee  [:,2]=a=it.t_vv
                # [:,3]=c=it.t_ve   (PE contracts over the d-chunk partitions)
                dots_ps = dpool.tile([TILE_P, 4], f32, tag="dots_ps")
                for c in range(N_CHUNKS):
                    nc.tensor.matmul(
                        dots_ps[:, 0:2], xp[:, c, :], th[:, c, :],
                        start=(c == 0), stop=(c == N_CHUNKS - 1),
                    )
                for c in range(N_CHUNKS):
                    nc.tensor.matmul(
                        dots_ps[:, 2:4], xp[:, N_CHUNKS + c, :],
                        th[:, N_CHUNKS + c, :],
                        start=(c == 0), stop=(c == N_CHUNKS - 1),
                    )
                dots = work.tile([TILE_P, 4], f32, tag="dots")
                nc.vector.tensor_copy(dots[:], dots_ps[:])
                d_b, d_d = dots[:, 0:1], dots[:, 1:2]
                d_a, d_c = dots[:, 2:3], dots[:, 3:4]

                # item-scaled products on ScalarE: p2 = it*b, p4 = it*d
                p2 = work.tile([TILE_P, D], f16, tag="p2")
                nc.scalar.activation(p2[:], it, AF.Copy, bias=0.0, scale=d_b)
                p4 = work.tile([TILE_P, D], f16, tag="p4")
                nc.scalar.activation(p4[:], it, AF.Copy, bias=0.0, scale=d_d)

                # fused scale+add on VectorE: v = u*a + p2, e = u*c + p4
                xo = outp.tile([TILE_P, 2, D], f16, tag="xo")
                nc.vector.scalar_tensor_tensor(
                    out=xo[:, 0, :], in0=u, scalar=d_a, in1=p2[:],
                    op0=OP.mult, op1=OP.add)
                nc.vector.scalar_tensor_tensor(
                    out=xo[:, 1, :], in0=u, scalar=d_c, in1=p4[:],
                    op0=OP.mult, op1=OP.add)
                if with_beta:
                    xo2 = outp.tile([TILE_P, 2, D], f16, tag="xo2")
                    nc.vector.tensor_add(xo2[:], xo[:], betas[:])
                    xo = xo2
                nc.sync.dma_start(xout_h[rows, :, :], xo[:])

    nc.compile()
    return nc


def _get_program(with_beta: bool):
    if with_beta not in _PROGRAM_CACHE:
        _PROGRAM_CACHE[with_beta] = _build_program(with_beta)
    return _PROGRAM_CACHE[with_beta]


def _prep_host_inputs(inputs):
    enc_user = np.asarray(inputs["enc_user"])
    enc_item = np.asarray(inputs["enc_item"])
    assert enc_user.shape == (B, D) and enc_item.shape == (B, D)

    xin = np.empty((B, 2 * D), dtype=np.float16)
    xin[:, :D] = enc_user
    xin[:, D:] = enc_item

    def vec(name):
        return np.asarray(inputs[name], dtype=np.float32).reshape(D)

    t_vv, t_ev = vec("theta_vv"), vec("theta_ev")
    t_ve, t_ee = vec("theta_ve"), vec("theta_ee")
    # th_pe[p, s, k]: s<8 -> u-dots thetas (t_ev, t_ee); s>=8 -> it-dots
    # thetas (t_vv, t_ve); d-index = (s % 8)*128 + p.
    th_pe = np.empty((TILE_P, 2 * N_CHUNKS, 2), dtype=np.float16)
    dgrid = t_vv.reshape(N_CHUNKS, TILE_P)  # [s, p] view helper
    th_pe[:, :N_CHUNKS, 0] = t_ev.reshape(N_CHUNKS, TILE_P).T
    th_pe[:, :N_CHUNKS, 1] = t_ee.reshape(N_CHUNKS, TILE_P).T
    th_pe[:, N_CHUNKS:, 0] = dgrid.T
    th_pe[:, N_CHUNKS:, 1] = t_ve.reshape(N_CHUNKS, TILE_P).T

    beta_v, beta_e = vec("beta_v"), vec("beta_e")
    with_beta = bool(np.any(beta_v) or np.any(beta_e))
    betas_b = None
    if with_beta:
        bb = np.stack([beta_v, beta_e]).astype(np.float16)  # [2, D]
        betas_b = np.ascontiguousarray(
            np.broadcast_to(bb[None, :, :], (TILE_P, 2, D))
        )
    return xin, th_pe, betas_b, with_beta


def _make_in_maps(xin, th_pe, betas_b, with_beta):
    in_maps = []
    for c in range(N_CORES):
        rows = slice(c * ROWS_PER_CORE, (c + 1) * ROWS_PER_CORE)
        m = {"xin": xin[rows], "th_pe": th_pe}
        if with_beta:
            m["betas"] = betas_b
        in_maps.append(m)
    return in_maps


def run_on_hw(inputs, trace=False):
    """Build/fetch the program, run it SPMD on 8 cores, gather outputs.

    Returns ((v_out, e_out), BassKernelResults).
    """
    import time

    from concourse.bass_utils import run_bass_kernel_spmd

    host = _prep_host_inputs(inputs)
    with_beta = host[-1]
    nc = _get_program(with_beta)
    in_maps = _make_in_maps(*host)
    for attempt in range(3):
        try:
            res = run_bass_kernel_spmd(nc, in_maps, list(range(N_CORES)), trace=trace)
            break
        except Exception:
            if attempt == 2:
                raise
            time.sleep(2.0)
    xout = np.concatenate(
        [np.asarray(res.results[c]["xout"]) for c in range(N_CORES)], axis=0
    )
    v = xout[:, 0, :].astype(np.float32)
    e = xout[:, 1, :].astype(np.float32)
    return (v, e), res


def kernel(**inputs):
    (v, e), _ = run_on_hw(inputs, trace=False)
    return v, e
